# revision 1
# baseline (speedup 1.0000x reference)
"""Child-Sum TreeLSTM over complete binary trees — Trainium2 Bass kernel.

Sharding: data-parallel over the batch-of-trees axis B=32 across 8 NeuronCores
(4 trees/core); the 8 gate weight matrices are replicated.

Per-core dataflow (activations kept feature-transposed in SBUF as
[feat-chunk(128,128,44), 3, cols] tiles; weights natural = lhsT):
  - level-by-level bottom-up; per <=512-column block:
      embs^T loaded by XBAR DMA-transpose directly from a host-prepared
      bf16 padded copy of embs ([..., 384] with feature 300 = 1.0 ones row)
      for levels >= 7; PE-transpose path for the small deep levels
      gate pre-acts accumulate in PSUM over 6 K-chunks: x-side bf16 +
      h-side float32r; the combined bias (bx+bh) rides as a 45th weight row
      against the baked-in ones row
      sigma/tanh evacuate PSUM->SBUF in one ACT instruction per gate
      per-child forget gates use a step-0 duplicated rhs (each parent column
      streamed twice) so fx lands directly at child granularity
      c_new = i*u + f1*c1 + f2*c2 and h = o*tanh(c) on DVE
      h^T -> PE-transpose -> natural -> DMA to output
  - levels 10/9/8 spill h^T/c^T through internal DRAM (SBUF pressure);
    levels <= 7 keep h^T/c^T resident in SBUF
  - matmuls with moving dim < 256 (deep levels) switch the h-side to bf16
    (float32r drops to 4 cycles/row below 256).
"""

import numpy as np
import ml_dtypes

import concourse.bass as bass
import concourse.mybir as mybir
import concourse.tile as tile
from concourse import bacc
from concourse.masks import make_identity
from concourse.bass_utils import run_bass_kernel_spmd

F32 = mybir.dt.float32
F32R = mybir.dt.float32r
BF16 = mybir.dt.bfloat16
AF = mybir.ActivationFunctionType

B, D, DIM = 32, 11, 300
N = 2**D - 1          # 2047
CORES = 8
BL = B // CORES       # trees per core
KS = [128, 128, 44]   # feature chunks of 300
KO = [0, 128, 256]
NBMAX = 512
SPILL_LV = (10, 9, 8)
SPOFF = {10: 0, 9: BL * 1024, 8: BL * 1024 + BL * 512}
SPTOT = BL * 1024 + BL * 512 + BL * 256
PROJ = {"i": 0, "f": 1, "o": 2, "u": 3}

_NC_CACHE = []


def _cols(l):
    return BL * (1 << l)


def _build():
    nc = bacc.Bacc("TRN2", target_bir_lowering=False, debug=False,
                   num_devices=CORES)
    embs = nc.dram_tensor("embs", [BL, N, DIM], F32, kind="ExternalInput")
    WX = nc.dram_tensor("wx", [128, 4, 3, DIM], F32R, kind="ExternalInput")
    WH = nc.dram_tensor("wh", [128, 4, 3, DIM], F32R, kind="ExternalInput")
    hout = nc.dram_tensor("hout", [BL, N, DIM], F32, kind="ExternalOutput")
    sph = nc.dram_tensor("sph", [128, 3, SPTOT], F32R)
    spc = nc.dram_tensor("spc", [128, 3, SPTOT], F32)

    with tile.TileContext(nc) as tc:
        import contextlib
        with contextlib.ExitStack() as ctx:
            sb = ctx.enter_context(tc.tile_pool(name="sb", bufs=1))
            exp = ctx.enter_context(tc.tile_pool(name="exp", bufs=2))
            xtp = ctx.enter_context(tc.tile_pool(name="xtp", bufs=2))
            hsp = ctx.enter_context(tc.tile_pool(name="hsp", bufs=2))
            gp = ctx.enter_context(tc.tile_pool(name="gp", bufs=5))
            fcp = ctx.enter_context(tc.tile_pool(name="fcp", bufs=2))
            onp = ctx.enter_context(tc.tile_pool(name="onp", bufs=2))
            hcb = ctx.enter_context(tc.tile_pool(name="hcb", bufs=2))
            rbp = ctx.enter_context(tc.tile_pool(name="rbp", bufs=2))
            stp = ctx.enter_context(tc.tile_pool(name="stp", bufs=1))
            psum = ctx.enter_context(
                tc.tile_pool(name="psum", bufs=2, space="PSUM"))

            ident = sb.tile([128, 128], F32)
            make_identity(nc, ident[:, :])

            wx_t = sb.tile([128, 4, 3, DIM], F32R, name="wx_t")
            wh_t = sb.tile([128, 4, 3, DIM], F32R, name="wh_t")
            nc.sync.dma_start(out=wx_t[:, :, :, :], in_=WX[:, :, :, :])
            nc.sync.dma_start(out=wh_t[:, :, :, :], in_=WH[:, :, :, :])
            wx = {nm: wx_t[:, p] for nm, p in PROJ.items()}
            wh = {nm: wh_t[:, p] for nm, p in PROJ.items()}

            def nat_ap(dram, l, r0, rs, w):
                base = (1 << l) - 1
                if l >= 7:
                    t, j0 = r0 >> l, r0 & ((1 << l) - 1)
                    return dram[t, base + j0: base + j0 + rs, 0:w]
                t0, tcnt = r0 >> l, rs >> l
                return dram[t0:t0 + tcnt, base:base + (1 << l), 0:w]

            def load_ex(l, c0, nb):
                """embs^T (+ones row) for parent cols [c0, c0+nb), bf16."""
                ex = exp.tile([128, 3, NBMAX], F32R, tag="ex")
                pT = psum.tile([128, 3, NBMAX], F32, tag="big")
                for r0 in range(0, nb, 128):
                    rs = min(128, nb - r0)
                    xt = xtp.tile([128, 304], F32, tag="xt")
                    nc.gpsimd.memset(xt[:, 300:304], 1.0)
                    nc.sync.dma_start(out=xt[0:rs, 0:300],
                                      in_=nat_ap(embs, l, c0 + r0, rs, DIM))
                    for f in range(3):
                        ke = KS[f] + (1 if f == 2 else 0)
                        nc.tensor.transpose(
                            out=pT[0:ke, f, r0:r0 + rs],
                            in_=xt[0:rs, KO[f]:KO[f] + ke],
                            identity=ident[0:rs, 0:rs])
                nc.scalar.copy(ex[0:128, 0, 0:nb], pT[0:128, 0, 0:nb])
                nc.scalar.copy(ex[0:128, 1, 0:nb], pT[0:128, 1, 0:nb])
                nc.scalar.copy(ex[0:45, 2, 0:nb], pT[0:45, 2, 0:nb])
                return ex

            def store_nat(l, c0, nb, hsrc, s0):
                for r0 in range(0, nb, 128):
                    rs = min(128, nb - r0)
                    pO = psum.tile([128, 304], F32, tag="oT")
                    for f in range(3):
                        nc.tensor.transpose(
                            out=pO[0:rs, KO[f]:KO[f] + KS[f]],
                            in_=hsrc[0:KS[f], f,
                                     s0 + r0:s0 + r0 + rs].bitcast(F32),
                            identity=ident[0:KS[f], 0:KS[f]])
                    on = onp.tile([128, 300], F32, tag="on")
                    nc.scalar.copy(on[0:rs, :], pO[0:rs, 0:300])
                    nc.gpsimd.dma_start(out=nat_ap(hout, l, c0 + r0, rs,
                                                   DIM),
                                        in_=on[0:rs, :])

            st_h = {l: stp.tile([128, 3, _cols(l)], F32R, tag=f"sh{l}",
                                name=f"sh{l}") for l in range(0, 8)}
            st_c = {l: stp.tile([128, 3, _cols(l)], F32, tag=f"sc{l}",
                                name=f"sc{l}") for l in range(0, 8)}

            # ---------------- leaves (level 10) ----------------
            def leaf_block(c0):
                l, nb = 10, NBMAX
                ex = load_ex(l, c0, nb)
                sg = {}
                for nm, fn in (("i", AF.Sigmoid), ("o", AF.Sigmoid),
                               ("u", AF.Tanh)):
                    pG = psum.tile([128, 3, NBMAX], F32, tag="big",
                                   name=f"lpg_{c0}_{nm}")
                    for m in range(3):
                        ms, mo = KS[m], KO[m]
                        for k in range(3):
                            kx = KS[k] + (1 if k == 2 else 0)
                            nc.tensor.matmul(
                                pG[0:ms, m, 0:nb],
                                wx[nm][0:kx, k, mo:mo + ms],
                                ex[0:kx, k, 0:nb],
                                start=(k == 0), stop=(k == 2))
                    g = gp.tile([128, 3, NBMAX], F32, tag="g",
                                name=f"lg_{c0}_{nm}")
                    nc.scalar.activation(g[:, :, 0:nb], pG[:, :, 0:nb], fn)
                    sg[nm] = g
                cb = hcb.tile([128, 3, NBMAX], F32, tag="lc", bufs=3,
                              name=f"lc_{c0}")
                hb = hcb.tile([128, 3, NBMAX], F32R, tag="lh", bufs=3,
                              name=f"lh_{c0}")
                nc.vector.tensor_mul(cb[:, :, 0:nb], sg["i"][:, :, 0:nb],
                                     sg["u"][:, :, 0:nb])
                th = gp.tile([128, 3, NBMAX], F32, tag="g",
                             name=f"lth_{c0}")
                nc.scalar.activation(th[:, :, 0:nb], cb[:, :, 0:nb], AF.Tanh)
                nc.vector.tensor_mul(hb[:, :, 0:nb], sg["o"][:, :, 0:nb],
                                     th[:, :, 0:nb])
                store_nat(l, c0, nb, hb, 0)
                return hb, cb

            # ---------------- internal levels 9..0 ----------------
            for l in range(9, -1, -1):
                cols = _cols(l)
                spill = l in SPILL_LV
                child_spill = (l + 1) in SPILL_LV
                for c0 in range(0, cols, NBMAX):
                    nb = min(NBMAX, cols - c0)
                    fs = min(2 * nb, NBMAX)
                    nsub = (2 * nb) // fs
                    if l == 9:
                        leaf_hc = [leaf_block(2 * c0 + s * fs)
                                   for s in range(nsub)]
                    ex = load_ex(l, c0, nb)

                    hn, cn = [], []
                    for s in range(nsub):
                        ch0 = 2 * c0 + s * fs
                        if l == 9:
                            hn.append((leaf_hc[s][0], 0))
                            cn.append((leaf_hc[s][1], 0))
                        elif child_spill:
                            t_h = rbp.tile([128, 3, NBMAX], F32R, tag="rh")
                            t_c = rbp.tile([128, 3, NBMAX], F32, tag="rc")
                            off = SPOFF[l + 1] + ch0
                            nc.sync.dma_start(out=t_h[:, :, 0:fs],
                                              in_=sph[:, :, off:off + fs])
                            nc.sync.dma_start(out=t_c[:, :, 0:fs],
                                              in_=spc[:, :, off:off + fs])
                            hn.append((t_h, 0))
                            cn.append((t_c, 0))
                        else:
                            hn.append((st_h[l + 1], ch0))
                            cn.append((st_c[l + 1], ch0))

                    hs = hsp.tile([128, 3, NBMAX], F32R, tag="hs",
                                  name=f"hs_{l}_{c0}")
                    for s in range(nsub):
                        t_h, o_h = hn[s]
                        pair = t_h[:, :, o_h:o_h + fs].rearrange(
                            "p c (n two) -> p c n two", two=2)
                        nc.vector.tensor_add(
                            hs[:, :, s * fs // 2:(s + 1) * fs // 2],
                            pair[:, :, :, 0], pair[:, :, :, 1])

                    sg = {}
                    for nm, fn in (("i", AF.Sigmoid), ("o", AF.Sigmoid),
                                   ("u", AF.Tanh)):
                        pG = psum.tile([128, 3, NBMAX], F32, tag="big")
                        for m in range(3):
                            ms, mo = KS[m], KO[m]
                            for k in range(3):
                                kx = KS[k] + (1 if k == 2 else 0)
                                nc.tensor.matmul(
                                    pG[0:ms, m, 0:nb],
                                    wx[nm][0:kx, k, mo:mo + ms],
                                    ex[0:kx, k, 0:nb],
                                    start=(k == 0), stop=False)
                            for k in range(3):
                                nc.tensor.matmul(
                                    pG[0:ms, m, 0:nb],
                                    wh[nm][0:KS[k], k, mo:mo + ms],
                                    hs[0:KS[k], k, 0:nb],
                                    start=False, stop=(k == 2))
                        g = gp.tile([128, 3, NBMAX], F32, tag="g")
                        nc.scalar.activation(g[:, :, 0:nb], pG[:, :, 0:nb], fn)
                        sg[nm] = g

                    if spill:
                        cdst = hcb.tile([128, 3, NBMAX], F32, tag="cb")
                        hdst = hcb.tile([128, 3, NBMAX], F32R, tag="hb")
                        d0 = 0
                    else:
                        cdst, hdst, d0 = st_c[l], st_h[l], c0

                    cc = cdst[:, :, d0:d0 + nb]
                    nc.vector.tensor_mul(cc, sg["i"][:, :, 0:nb],
                                         sg["u"][:, :, 0:nb])

                    for s in range(nsub):
                        pF = psum.tile([128, 3, NBMAX], F32, tag="big")
                        p0 = s * fs // 2
                        w_h = wh["f"]
                        t_h, o_h = hn[s]
                        for m in range(3):
                            ms, mo = KS[m], KO[m]
                            for k in range(3):
                                kx = KS[k] + (1 if k == 2 else 0)
                                dup = ex[0:kx, k, p0:p0 + fs // 2] \
                                    .unsqueeze(2).broadcast_to([kx, fs // 2, 2])
                                nc.tensor.matmul(
                                    pF[0:ms, m, 0:fs],
                                    wx["f"][0:kx, k, mo:mo + ms], dup,
                                    start=(k == 0), stop=False)
                            for k in range(3):
                                nc.tensor.matmul(
                                    pF[0:ms, m, 0:fs],
                                    w_h[0:KS[k], k, mo:mo + ms],
                                    t_h[0:KS[k], k, o_h:o_h + fs],
                                    start=False, stop=(k == 2))
                        fg = gp.tile([128, 3, NBMAX], F32, tag="g")
                        nc.scalar.activation(fg[:, :, 0:fs], pF[:, :, 0:fs],
                                             AF.Sigmoid)
                        t_c, o_c = cn[s]
                        fc = fcp.tile([128, 3, NBMAX], F32, tag="fc")
                        nc.vector.tensor_mul(fc[:, :, 0:fs],
                                             fg[:, :, 0:fs],
                                             t_c[:, :, o_c:o_c + fs])
                        pair = fc[:, :, 0:fs].rearrange(
                            "p c (n two) -> p c n two", two=2)
                        ccs = cdst[:, :, d0 + p0:d0 + p0 + fs // 2]
                        nc.vector.tensor_add(ccs, ccs, pair[:, :, :, 0])
                        nc.vector.tensor_add(ccs, ccs, pair[:, :, :, 1])

                    th = gp.tile([128, 3, NBMAX], F32, tag="g")
                    nc.scalar.activation(th[:, :, 0:nb], cc, AF.Tanh)
                    nc.vector.tensor_mul(hdst[:, :, d0:d0 + nb],
                                         sg["o"][:, :, 0:nb], th[:, :, 0:nb])

                    if spill:
                        off = SPOFF[l] + c0
                        nc.gpsimd.dma_start(out=sph[:, :, off:off + nb],
                                            in_=hdst[:, :, 0:nb])
                        nc.gpsimd.dma_start(out=spc[:, :, off:off + nb],
                                            in_=cdst[:, :, 0:nb])
                    store_nat(l, c0, nb, hdst, d0)
    nc.compile()
    return nc


def kernel(embs, Wix, bix, Wih, bih, Wfx, bfx, Wfh, bfh,
           Wox, box, Woh, boh, Wux, bux, Wuh, buh):
    embs = np.ascontiguousarray(np.asarray(embs, dtype=np.float32))
    if not _NC_CACHE:
        _NC_CACHE.append(_build())
    nc = _NC_CACHE[0]

    def chunked(stack, bias_rows):
        out = np.zeros((128, 4, 3, DIM), np.float32)
        for p in range(4):
            out[0:128, p, 0] = stack[p][0:128]
            out[0:128, p, 1] = stack[p][128:256]
            out[0:44, p, 2] = stack[p][256:300]
            if bias_rows is not None:
                out[44, p, 2] = bias_rows[p]
        return out

    xw = [np.asarray(w, np.float32) for w in (Wix, Wfx, Wox, Wux)]
    xb = [np.asarray(bix) + np.asarray(bih), np.asarray(bfx) + np.asarray(bfh),
          np.asarray(box) + np.asarray(boh), np.asarray(bux) + np.asarray(buh)]
    hw_ = [np.asarray(w, np.float32) for w in (Wih, Wfh, Woh, Wuh)]
    wxp = chunked(xw, xb)
    whp = chunked(hw_, None)

    in_maps = [{"embs": embs[c * BL:(c + 1) * BL],
                "wx": wxp, "wh": whp}
               for c in range(CORES)]
    res = run_bass_kernel_spmd(nc, in_maps, list(range(CORES)))
    return np.concatenate([res.results[c]["hout"] for c in range(CORES)],
                          axis=0)



# revision 2
# speedup vs baseline: 1.3015x; 1.3015x over previous
"""Child-Sum TreeLSTM over complete binary trees — Trainium2 Bass kernel (v2).

Sharding: data-parallel over batch B=32 across 8 cores (4 trees/core),
weights replicated.

v2 design (vs v1): all-bf16 datapath, zero on-device transposes.
  - Host pre-transposes embs into feature-major layout [128, 3, 8188] bf16
    with a baked ones-row (feature slot 300) riding the k=2 chunk; the
    combined bias (bx+bh) is a 45th weight row.
  - Weight slabs WXS/WHS [128, 3kc, 12grp, 128] bf16 (gate x m-chunk grid).
  - All h/c state for all 11 levels stays resident in SBUF as bf16
    [128, 3, cols] tiles — no DRAM spills.
  - Gate pre-acts accumulate in PSUM f32 over 3 x-chunks + 3 h-chunks;
    one Act instruction per gate evacuates PSUM -> SBUF bf16 with sigma/tanh.
  - Per-child forget gates duplicate each parent ex column (broadcast AP).
  - c/h math on DVE in bf16; h written transposed to DRAM bf16, host
    un-transposes and upcasts to f32.
"""

import numpy as np
import ml_dtypes

import concourse.bass as bass
import concourse.mybir as mybir
import concourse.tile as tile
from concourse import bacc
from concourse.bass_utils import run_bass_kernel_spmd

F32 = mybir.dt.float32
BF16 = mybir.dt.bfloat16
AF = mybir.ActivationFunctionType

B, D, DIM = 32, 11, 300
N = 2**D - 1          # 2047
CORES = 8
BL = B // CORES       # 4 trees per core
NTOT = BL * N         # 8188 columns per core
KS = [128, 128, 44]   # feature chunks of 300
NB = 512
GATES = ("i", "o", "u", "f")

# level -> column offset in the level-major layout (leaves first)
OFF = {}
_o = 0
for _l in range(D - 1, -1, -1):
    OFF[_l] = _o
    _o += BL * (1 << _l)
TAIL_LV = 7           # levels <= TAIL_LV use the resident ex tile
TOFF = OFF[TAIL_LV]   # 7168
TCOLS = NTOT - TOFF   # 1020

_NC_CACHE = []


def _cols(l):
    return BL * (1 << l)


def _build():
    nc = bacc.Bacc("TRN2", target_bir_lowering=False, debug=False,
                   num_devices=CORES)
    exT = nc.dram_tensor("ext", [128, 3, NTOT], BF16, kind="ExternalInput")
    WXS = nc.dram_tensor("wxs", [128, 3, 12, 128], BF16, kind="ExternalInput")
    WHS = nc.dram_tensor("whs", [128, 3, 12, 128], BF16, kind="ExternalInput")
    houtT = nc.dram_tensor("houtt", [128, 3, NTOT], BF16,
                           kind="ExternalOutput")

    with tile.TileContext(nc) as tc:
        import contextlib
        with contextlib.ExitStack() as ctx:
            sb = ctx.enter_context(tc.tile_pool(name="sb", bufs=1))
            exp = ctx.enter_context(tc.tile_pool(name="exp", bufs=3))
            hsp = ctx.enter_context(tc.tile_pool(name="hsp", bufs=2))
            gp = ctx.enter_context(tc.tile_pool(name="gp", bufs=6))
            fcp = ctx.enter_context(tc.tile_pool(name="fcp", bufs=2))
            stp = ctx.enter_context(tc.tile_pool(name="stp", bufs=1))
            psum = ctx.enter_context(
                tc.tile_pool(name="psum", bufs=2, space="PSUM"))

            wx_t = sb.tile([128, 3, 12, 128], BF16, name="wx_t")
            wh_t = sb.tile([128, 3, 12, 128], BF16, name="wh_t")
            nc.sync.dma_start(out=wx_t[:, :, :, :], in_=WXS[:, :, :, :])
            nc.sync.dma_start(out=wh_t[:, :, :, :], in_=WHS[:, :, :, :])

            ex_tail = sb.tile([128, 3, TCOLS], BF16, name="ex_tail")
            nc.sync.dma_start(out=ex_tail[:, :, :],
                              in_=exT[:, :, TOFF:TOFF + TCOLS])

            # persistent per-level h/c state (bf16, feature-transposed)
            st_h = {l: stp.tile([128, 3, _cols(l)], BF16, tag=f"sh{l}",
                                name=f"sh{l}") for l in range(D)}
            st_c = {l: stp.tile([128, 3, _cols(l)], BF16, tag=f"sc{l}",
                                name=f"sc{l}") for l in range(D)}

            def gate_x_matmuls(pG, g, ex, e0, nb, start, stop):
                """Accumulate x-side pre-act for gate g over cols [e0,e0+nb)."""
                for m in range(3):
                    ms = KS[m]
                    for k in range(3):
                        kx = KS[k] + (1 if k == 2 else 0)
                        nc.tensor.matmul(
                            pG[0:ms, m, 0:nb],
                            wx_t[0:kx, k, 3 * GATES.index(g) + m, 0:ms],
                            ex[0:kx, k, e0:e0 + nb],
                            start=(start and k == 0),
                            stop=(stop and k == 2))

            def gate_h_matmuls(pG, g, hs, h0, nb, stop):
                for m in range(3):
                    ms = KS[m]
                    for k in range(3):
                        nc.tensor.matmul(
                            pG[0:ms, m, 0:nb],
                            wh_t[0:KS[k], k, 3 * GATES.index(g) + m, 0:ms],
                            hs[0:KS[k], k, h0:h0 + nb],
                            start=False, stop=(stop and k == 2))

            def ex_for(l, c0, nb):
                """Return (tile, base offset) holding ex cols of level l."""
                if l <= TAIL_LV:
                    return ex_tail, OFF[l] - TOFF + c0
                t = exp.tile([128, 3, NB], BF16, tag="ex")
                nc.sync.dma_start(
                    out=t[:, :, 0:nb],
                    in_=exT[:, :, OFF[l] + c0:OFF[l] + c0 + nb])
                return t, 0

            # ---------------- leaves (level 10) ----------------
            l = D - 1
            for c0 in range(0, _cols(l), NB):
                nb = min(NB, _cols(l) - c0)
                ex, e0 = ex_for(l, c0, nb)
                sg = {}
                for g, fn in (("i", AF.Sigmoid), ("o", AF.Sigmoid),
                              ("u", AF.Tanh)):
                    pG = psum.tile([128, 3, NB], F32, tag="big")
                    gate_x_matmuls(pG, g, ex, e0, nb, True, True)
                    gt = gp.tile([128, 3, NB], BF16, tag="g")
                    nc.scalar.activation(gt[:, :, 0:nb], pG[:, :, 0:nb], fn)
                    sg[g] = gt
                cc = st_c[l][:, :, c0:c0 + nb]
                nc.vector.tensor_mul(cc, sg["i"][:, :, 0:nb],
                                     sg["u"][:, :, 0:nb])
                th = gp.tile([128, 3, NB], BF16, tag="g")
                nc.scalar.activation(th[:, :, 0:nb], cc, AF.Tanh)
                nc.vector.tensor_mul(st_h[l][:, :, c0:c0 + nb],
                                     sg["o"][:, :, 0:nb], th[:, :, 0:nb])
            nc.sync.dma_start(out=houtT[:, :, OFF[l]:OFF[l] + _cols(l)],
                              in_=st_h[l][:, :, :])

            # ---------------- internal levels 9..0 ----------------
            for l in range(D - 2, -1, -1):
                cols = _cols(l)
                for c0 in range(0, cols, NB):
                    nb = min(NB, cols - c0)
                    fs = min(2 * nb, NB)     # child-block width
                    nsub = (2 * nb) // fs
                    ex, e0 = ex_for(l, c0, nb)
                    ch_h = st_h[l + 1]
                    ch_c = st_c[l + 1]

                    # child sum h1+h2 -> hs (bf16)
                    hs = hsp.tile([128, 3, NB], BF16, tag="hs")
                    pair = ch_h[:, :, 2 * c0:2 * c0 + 2 * nb].rearrange(
                        "p c (n two) -> p c n two", two=2)
                    nc.vector.tensor_add(hs[:, :, 0:nb],
                                         pair[:, :, :, 0], pair[:, :, :, 1])

                    sg = {}
                    for g, fn in (("i", AF.Sigmoid), ("o", AF.Sigmoid),
                                  ("u", AF.Tanh)):
                        pG = psum.tile([128, 3, NB], F32, tag="big")
                        gate_x_matmuls(pG, g, ex, e0, nb, True, False)
                        gate_h_matmuls(pG, g, hs, 0, nb, True)
                        gt = gp.tile([128, 3, NB], BF16, tag="g")
                        nc.scalar.activation(gt[:, :, 0:nb], pG[:, :, 0:nb],
                                             fn)
                        sg[g] = gt

                    cc = st_c[l][:, :, c0:c0 + nb]
                    nc.vector.tensor_mul(cc, sg["i"][:, :, 0:nb],
                                         sg["u"][:, :, 0:nb])

                    for s in range(nsub):
                        ch0 = 2 * c0 + s * fs
                        p0 = s * fs // 2
                        pF = psum.tile([128, 3, NB], F32, tag="big")
                        for m in range(3):
                            ms = KS[m]
                            for k in range(3):
                                kx = KS[k] + (1 if k == 2 else 0)
                                dup = ex[0:kx, k, e0 + p0:e0 + p0 + fs // 2] \
                                    .unsqueeze(2).broadcast_to(
                                        [kx, fs // 2, 2])
                                nc.tensor.matmul(
                                    pF[0:ms, m, 0:fs],
                                    wx_t[0:kx, k, 9 + m, 0:ms],
                                    dup, start=(k == 0), stop=False)
                            for k in range(3):
                                nc.tensor.matmul(
                                    pF[0:ms, m, 0:fs],
                                    wh_t[0:KS[k], k, 9 + m, 0:ms],
                                    ch_h[0:KS[k], k, ch0:ch0 + fs],
                                    start=False, stop=(k == 2))
                        fg = gp.tile([128, 3, NB], BF16, tag="g")
                        nc.scalar.activation(fg[:, :, 0:fs], pF[:, :, 0:fs],
                                             AF.Sigmoid)
                        fc = fcp.tile([128, 3, NB], BF16, tag="fc")
                        nc.vector.tensor_mul(fc[:, :, 0:fs],
                                             fg[:, :, 0:fs],
                                             ch_c[:, :, ch0:ch0 + fs])
                        fpair = fc[:, :, 0:fs].rearrange(
                            "p c (n two) -> p c n two", two=2)
                        ccs = cc[:, :, p0:p0 + fs // 2] if nsub > 1 else cc
                        nc.vector.tensor_add(ccs, ccs, fpair[:, :, :, 0])
                        nc.vector.tensor_add(ccs, ccs, fpair[:, :, :, 1])

                    th = gp.tile([128, 3, NB], BF16, tag="g")
                    nc.scalar.activation(th[:, :, 0:nb], cc, AF.Tanh)
                    nc.vector.tensor_mul(st_h[l][:, :, c0:c0 + nb],
                                         sg["o"][:, :, 0:nb], th[:, :, 0:nb])
                nc.sync.dma_start(out=houtT[:, :, OFF[l]:OFF[l] + cols],
                                  in_=st_h[l][:, :, :])
    nc.compile()
    return nc


def _prep_inputs(embs, Wx, bx, Wh, bh):
    """Host-side: transposed bf16 ex + weight slabs."""
    bf = ml_dtypes.bfloat16
    ex = np.zeros((CORES, 128, 3, NTOT), dtype=bf)
    e32 = np.asarray(embs, np.float32)
    for c in range(CORES):
        ec = e32[BL * c:BL * (c + 1)]          # [BL, N, 300]
        for l in range(D - 1, -1, -1):
            n0, n1 = (1 << l) - 1, (1 << (l + 1)) - 1
            T = ec[:, n0:n1, :].reshape(BL * (1 << l), DIM).T  # [300, cols]
            o0 = OFF[l]
            nbl = BL * (1 << l)
            ex[c, :, 0, o0:o0 + nbl] = T[0:128]
            ex[c, :, 1, o0:o0 + nbl] = T[128:256]
            ex[c, 0:44, 2, o0:o0 + nbl] = T[256:300]
            ex[c, 44, 2, o0:o0 + nbl] = 1.0

    def slab(Ws, biases):
        out = np.zeros((128, 3, 12, 128), dtype=bf)
        for gi in range(4):
            W = np.asarray(Ws[gi], np.float32)
            for m in range(3):
                ms = KS[m]
                blk = W[:, 128 * m:128 * m + ms]           # [300, ms]
                g = 3 * gi + m
                out[:, 0, g, 0:ms] = blk[0:128]
                out[:, 1, g, 0:ms] = blk[128:256]
                out[0:44, 2, g, 0:ms] = blk[256:300]
                if biases is not None:
                    out[44, 2, g, 0:ms] = biases[gi][128 * m:128 * m + ms]
        return out

    wxs = slab(Wx, bx)
    whs = slab(Wh, None)
    return ex, wxs, whs


def kernel(embs, Wix, bix, Wih, bih, Wfx, bfx, Wfh, bfh,
           Wox, box, Woh, boh, Wux, bux, Wuh, buh):
    if not _NC_CACHE:
        _NC_CACHE.append(_build())
    nc = _NC_CACHE[0]

    bxs = [np.asarray(bix) + np.asarray(bih),
           np.asarray(box) + np.asarray(boh),
           np.asarray(bux) + np.asarray(buh),
           np.asarray(bfx) + np.asarray(bfh)]
    ex, wxs, whs = _prep_inputs(embs, [Wix, Wox, Wux, Wfx], bxs,
                                [Wih, Woh, Wuh, Wfh], None)

    in_maps = [{"ext": ex[c], "wxs": wxs, "whs": whs} for c in range(CORES)]
    res = run_bass_kernel_spmd(nc, in_maps, list(range(CORES)))

    hout = np.zeros((B, N, DIM), np.float32)
    for c in range(CORES):
        ht = np.asarray(res.results[c]["houtt"], np.float32)  # [128,3,NTOT]
        for l in range(D):
            n0, n1 = (1 << l) - 1, (1 << (l + 1)) - 1
            nbl = BL * (1 << l)
            o0 = OFF[l]
            Hl = np.concatenate(
                [ht[0:128, 0, o0:o0 + nbl], ht[0:128, 1, o0:o0 + nbl],
                 ht[0:44, 2, o0:o0 + nbl]], axis=0)         # [300, cols]
            hout[BL * c:BL * (c + 1), n0:n1, :] = \
                Hl.T.reshape(BL, 1 << l, DIM)
    return hout


# revision 3
# speedup vs baseline: 1.4473x; 1.1120x over previous
"""Child-Sum TreeLSTM over complete binary trees — Trainium2 Bass kernel (v2).

Sharding: data-parallel over batch B=32 across 8 cores (4 trees/core),
weights replicated.

v2 design (vs v1): all-bf16 datapath, zero on-device transposes.
  - Host pre-transposes embs into feature-major layout [128, 3, 8188] bf16
    with a baked ones-row (feature slot 300) riding the k=2 chunk; the
    combined bias (bx+bh) is a 45th weight row.
  - Weight slabs WXS/WHS [128, 3kc, 12grp, 128] bf16 (gate x m-chunk grid).
  - All h/c state for all 11 levels stays resident in SBUF as bf16
    [128, 3, cols] tiles — no DRAM spills.
  - Gate pre-acts accumulate in PSUM f32 over 3 x-chunks + 3 h-chunks;
    one Act instruction per gate evacuates PSUM -> SBUF bf16 with sigma/tanh.
  - Per-child forget gates duplicate each parent ex column (broadcast AP).
  - c/h math on DVE in bf16; h written transposed to DRAM bf16, host
    un-transposes and upcasts to f32.
"""

import numpy as np
import ml_dtypes

import concourse.bass as bass
import concourse.mybir as mybir
import concourse.tile as tile
from concourse import bacc
from concourse.bass_utils import run_bass_kernel_spmd

F32 = mybir.dt.float32
F32R = mybir.dt.float32r
BF16 = mybir.dt.bfloat16
AF = mybir.ActivationFunctionType

B, D, DIM = 32, 11, 300
N = 2**D - 1          # 2047
CORES = 8
BL = B // CORES       # 4 trees per core
NTOT = BL * N         # 8188 columns per core
KS = [128, 128, 44]   # feature chunks of 300
NB = 512
GATES = ("i", "o", "u", "f")

# level -> column offset in the level-major layout (leaves first)
OFF = {}
_o = 0
for _l in range(D - 1, -1, -1):
    OFF[_l] = _o
    _o += BL * (1 << _l)
TAIL_LV = 7           # levels <= TAIL_LV use the resident ex tile
TOFF = OFF[TAIL_LV]   # 7168
TCOLS = NTOT - TOFF   # 1020

_NC_CACHE = []


def _cols(l):
    return BL * (1 << l)


def _build():
    nc = bacc.Bacc("TRN2", target_bir_lowering=False, debug=False,
                   num_devices=CORES)
    exT = nc.dram_tensor("ext", [128, 3, NTOT], BF16, kind="ExternalInput")
    WXS = nc.dram_tensor("wxs", [128, 3, 12, 128], BF16, kind="ExternalInput")
    WHS = nc.dram_tensor("whs", [128, 3, 12, 128], BF16, kind="ExternalInput")
    houtT = nc.dram_tensor("houtt", [128, 3, NTOT], BF16,
                           kind="ExternalOutput")

    with tile.TileContext(nc) as tc:
        import contextlib
        with contextlib.ExitStack() as ctx:
            sb = ctx.enter_context(tc.tile_pool(name="sb", bufs=1))
            exp = ctx.enter_context(tc.tile_pool(name="exp", bufs=6))
            hsp = ctx.enter_context(tc.tile_pool(name="hsp", bufs=2))
            gp = ctx.enter_context(tc.tile_pool(name="gp", bufs=8))
            fcp = ctx.enter_context(tc.tile_pool(name="fcp", bufs=2))
            stp = ctx.enter_context(tc.tile_pool(name="stp", bufs=1))
            psum = ctx.enter_context(
                tc.tile_pool(name="psum", bufs=2, space="PSUM"))

            wx_t = sb.tile([128, 3, 12, 128], BF16, name="wx_t")
            wh_t = sb.tile([128, 3, 12, 128], BF16, name="wh_t")
            ex_tail = sb.tile([128, 3, TCOLS], BF16, name="ex_tail")

            # persistent per-level h/c state (bf16, feature-transposed)
            st_h = {l: stp.tile([128, 3, _cols(l)], BF16, tag=f"sh{l}",
                                name=f"sh{l}") for l in range(D)}
            st_c = {l: stp.tile([128, 3, _cols(l)], BF16, tag=f"sc{l}",
                                name=f"sc{l}") for l in range(D)}

            def gate_x_matmuls(pG, g, ex, e0, nb, start, stop):
                """Accumulate x-side pre-act for gate g over cols [e0,e0+nb)."""
                for m in range(3):
                    ms = KS[m]
                    for k in range(3):
                        kx = KS[k] + (1 if k == 2 else 0)
                        nc.tensor.matmul(
                            pG[0:ms, m, 0:nb],
                            wx_t[0:kx, k, 3 * GATES.index(g) + m, 0:ms],
                            ex[0:kx, k, e0:e0 + nb],
                            start=(start and k == 0),
                            stop=(stop and k == 2))

            def gate_h_matmuls(pG, g, hs, h0, nb, stop):
                for m in range(3):
                    ms = KS[m]
                    for k in range(3):
                        nc.tensor.matmul(
                            pG[0:ms, m, 0:nb],
                            wh_t[0:KS[k], k, 3 * GATES.index(g) + m, 0:ms],
                            hs[0:KS[k], k, h0:h0 + nb],
                            start=False, stop=(stop and k == 2))

            def ex_for(l, c0, nb):
                """Return (tile, base offset) holding ex cols of level l."""
                if l <= TAIL_LV:
                    return ex_tail, OFF[l] - TOFF + c0
                t = exp.tile([128, 3, NB], BF16, tag="ex")
                nc.sync.dma_start(
                    out=t[:, :, 0:nb],
                    in_=exT[:, :, OFF[l] + c0:OFF[l] + c0 + nb])
                return t, 0

            def blocks_of(l):
                cols = _cols(l)
                if cols > NB:
                    return NB
                return max(cols // 2, 2) if cols >= 16 else cols

            # ---------------- leaves (level 10) ----------------
            l = D - 1
            ex0, e00 = ex_for(l, 0, NB)
            nc.sync.dma_start(out=wx_t[:, :, :, :], in_=WXS[:, :, :, :])
            first_leaf = True
            for c0 in range(0, _cols(l), NB):
                nb = min(NB, _cols(l) - c0)
                if first_leaf:
                    ex, e0 = ex0, e00
                    first_leaf = False
                else:
                    ex, e0 = ex_for(l, c0, nb)
                sg = {}
                for g, fn in (("i", AF.Sigmoid), ("u", AF.Tanh),
                              ("o", AF.Sigmoid)):
                    pG = psum.tile([128, 3, NB], F32, tag="big")
                    gate_x_matmuls(pG, g, ex, e0, nb, True, True)
                    gt = gp.tile([128, 3, NB], BF16, tag="g")
                    nc.scalar.activation(gt[:, :, 0:nb], pG[:, :, 0:nb], fn)
                    sg[g] = gt
                    if g == "u":
                        cc = st_c[l][:, :, c0:c0 + nb]
                        nc.vector.tensor_mul(cc, sg["i"][:, :, 0:nb],
                                             sg["u"][:, :, 0:nb])
                if c0 == 0:
                    nc.sync.dma_start(out=wh_t[:, :, :, :],
                                      in_=WHS[:, :, :, :])
                th = gp.tile([128, 3, NB], BF16, tag="g")
                nc.scalar.activation(th[:, :, 0:nb], cc, AF.Tanh)
                nc.vector.tensor_mul(st_h[l][:, :, c0:c0 + nb],
                                     sg["o"][:, :, 0:nb], th[:, :, 0:nb])
            nc.sync.dma_start(out=ex_tail[:, :, :],
                              in_=exT[:, :, TOFF:TOFF + TCOLS])
            nc.gpsimd.dma_start(out=houtT[:, :, OFF[l]:OFF[l] + _cols(l)],
                                in_=st_h[l][:, :, :])

            # ---------------- internal levels 9..0 ----------------
            for l in range(D - 2, -1, -1):
                cols = _cols(l)
                nbl = blocks_of(l)
                for c0 in range(0, cols, nbl):
                    nb = min(nbl, cols - c0)
                    fs = min(2 * nb, NB)     # child-block width
                    nsub = (2 * nb) // fs
                    ex, e0 = ex_for(l, c0, nb)
                    ch_h = st_h[l + 1]
                    ch_c = st_c[l + 1]

                    # forget gates first: they depend only on child h/c
                    pFs = []
                    for s in range(nsub):
                        ch0 = 2 * c0 + s * fs
                        p0 = s * fs // 2
                        pF = psum.tile([128, 3, NB], F32, tag="big")
                        for m in range(3):
                            ms = KS[m]
                            for k in range(3):
                                kx = KS[k] + (1 if k == 2 else 0)
                                dup = ex[0:kx, k, e0 + p0:e0 + p0 + fs // 2] \
                                    .unsqueeze(2).broadcast_to(
                                        [kx, fs // 2, 2])
                                nc.tensor.matmul(
                                    pF[0:ms, m, 0:fs],
                                    wx_t[0:kx, k, 9 + m, 0:ms],
                                    dup, start=(k == 0), stop=False)
                            for k in range(3):
                                nc.tensor.matmul(
                                    pF[0:ms, m, 0:fs],
                                    wh_t[0:KS[k], k, 9 + m, 0:ms],
                                    ch_h[0:KS[k], k, ch0:ch0 + fs],
                                    start=False, stop=(k == 2))
                        fg = gp.tile([128, 3, NB], BF16, tag="g")
                        nc.scalar.activation(fg[:, :, 0:fs], pF[:, :, 0:fs],
                                             AF.Sigmoid)
                        pFs.append((fg, s))

                    # child sum h1+h2 -> hs (bf16)
                    hs = hsp.tile([128, 3, NB], BF16, tag="hs")
                    pair = ch_h[:, :, 2 * c0:2 * c0 + 2 * nb].rearrange(
                        "p c (n two) -> p c n two", two=2)
                    nc.vector.tensor_add(hs[:, :, 0:nb],
                                         pair[:, :, :, 0], pair[:, :, :, 1])

                    sg = {}
                    cc = st_c[l][:, :, c0:c0 + nb]
                    for g, fn in (("i", AF.Sigmoid), ("u", AF.Tanh),
                                  ("o", AF.Sigmoid)):
                        pG = psum.tile([128, 3, NB], F32, tag="big")
                        gate_x_matmuls(pG, g, ex, e0, nb, True, False)
                        gate_h_matmuls(pG, g, hs, 0, nb, True)
                        gt = gp.tile([128, 3, NB], BF16, tag="g")
                        nc.scalar.activation(gt[:, :, 0:nb], pG[:, :, 0:nb],
                                             fn)
                        sg[g] = gt
                        if g == "u":
                            nc.vector.tensor_mul(cc, sg["i"][:, :, 0:nb],
                                                 sg["u"][:, :, 0:nb])

                    for fg, s in pFs:
                        ch0 = 2 * c0 + s * fs
                        p0 = s * fs // 2
                        fc = fcp.tile([128, 3, NB], BF16, tag="fc")
                        nc.vector.tensor_mul(fc[:, :, 0:fs],
                                             fg[:, :, 0:fs],
                                             ch_c[:, :, ch0:ch0 + fs])
                        fpair = fc[:, :, 0:fs].rearrange(
                            "p c (n two) -> p c n two", two=2)
                        ccs = cc[:, :, p0:p0 + fs // 2] if nsub > 1 else cc
                        nc.vector.tensor_add(ccs, ccs, fpair[:, :, :, 0])
                        nc.vector.tensor_add(ccs, ccs, fpair[:, :, :, 1])

                    th = gp.tile([128, 3, NB], BF16, tag="g")
                    nc.scalar.activation(th[:, :, 0:nb], cc, AF.Tanh)
                    nc.vector.tensor_mul(st_h[l][:, :, c0:c0 + nb],
                                         sg["o"][:, :, 0:nb], th[:, :, 0:nb])
                nc.gpsimd.dma_start(out=houtT[:, :, OFF[l]:OFF[l] + cols],
                                    in_=st_h[l][:, :, :])
    nc.compile()
    return nc


def _prep_inputs(embs, Wx, bx, Wh, bh):
    """Host-side: transposed bf16 ex + weight slabs."""
    bf = ml_dtypes.bfloat16
    ex = np.zeros((CORES, 128, 3, NTOT), dtype=bf)
    e32 = np.asarray(embs, np.float32)
    for c in range(CORES):
        ec = e32[BL * c:BL * (c + 1)]          # [BL, N, 300]
        for l in range(D - 1, -1, -1):
            n0, n1 = (1 << l) - 1, (1 << (l + 1)) - 1
            T = ec[:, n0:n1, :].reshape(BL * (1 << l), DIM).T  # [300, cols]
            o0 = OFF[l]
            nbl = BL * (1 << l)
            ex[c, :, 0, o0:o0 + nbl] = T[0:128]
            ex[c, :, 1, o0:o0 + nbl] = T[128:256]
            ex[c, 0:44, 2, o0:o0 + nbl] = T[256:300]
            ex[c, 44, 2, o0:o0 + nbl] = 1.0

    def slab(Ws, biases):
        out = np.zeros((128, 3, 12, 128), dtype=bf)
        for gi in range(4):
            W = np.asarray(Ws[gi], np.float32)
            for m in range(3):
                ms = KS[m]
                blk = W[:, 128 * m:128 * m + ms]           # [300, ms]
                g = 3 * gi + m
                out[:, 0, g, 0:ms] = blk[0:128]
                out[:, 1, g, 0:ms] = blk[128:256]
                out[0:44, 2, g, 0:ms] = blk[256:300]
                if biases is not None:
                    out[44, 2, g, 0:ms] = biases[gi][128 * m:128 * m + ms]
        return out

    wxs = slab(Wx, bx)
    whs = slab(Wh, None)
    return ex, wxs, whs


def kernel(embs, Wix, bix, Wih, bih, Wfx, bfx, Wfh, bfh,
           Wox, box, Woh, boh, Wux, bux, Wuh, buh):
    if not _NC_CACHE:
        _NC_CACHE.append(_build())
    nc = _NC_CACHE[0]

    bxs = [np.asarray(bix) + np.asarray(bih),
           np.asarray(box) + np.asarray(boh),
           np.asarray(bux) + np.asarray(buh),
           np.asarray(bfx) + np.asarray(bfh)]
    ex, wxs, whs = _prep_inputs(embs, [Wix, Wox, Wux, Wfx], bxs,
                                [Wih, Woh, Wuh, Wfh], None)

    in_maps = [{"ext": ex[c], "wxs": wxs, "whs": whs} for c in range(CORES)]
    res = run_bass_kernel_spmd(nc, in_maps, list(range(CORES)))

    hout = np.zeros((B, N, DIM), np.float32)
    for c in range(CORES):
        ht = np.asarray(res.results[c]["houtt"], np.float32)  # [128,3,NTOT]
        for l in range(D):
            n0, n1 = (1 << l) - 1, (1 << (l + 1)) - 1
            nbl = BL * (1 << l)
            o0 = OFF[l]
            Hl = np.concatenate(
                [ht[0:128, 0, o0:o0 + nbl], ht[0:128, 1, o0:o0 + nbl],
                 ht[0:44, 2, o0:o0 + nbl]], axis=0)         # [300, cols]
            hout[BL * c:BL * (c + 1), n0:n1, :] = \
                Hl.T.reshape(BL, 1 << l, DIM)
    return hout


# revision 4
# speedup vs baseline: 1.4643x; 1.0118x over previous
"""Child-Sum TreeLSTM over complete binary trees — Trainium2 Bass kernel (v2).

Sharding: data-parallel over batch B=32 across 8 cores (4 trees/core),
weights replicated.

v2 design (vs v1): all-bf16 datapath, zero on-device transposes.
  - Host pre-transposes embs into feature-major layout [128, 3, 8188] bf16
    with a baked ones-row (feature slot 300) riding the k=2 chunk; the
    combined bias (bx+bh) is a 45th weight row.
  - Weight slabs WXS/WHS [128, 3kc, 12grp, 128] bf16 (gate x m-chunk grid).
  - All h/c state for all 11 levels stays resident in SBUF as bf16
    [128, 3, cols] tiles — no DRAM spills.
  - Gate pre-acts accumulate in PSUM f32 over 3 x-chunks + 3 h-chunks;
    one Act instruction per gate evacuates PSUM -> SBUF bf16 with sigma/tanh.
  - Per-child forget gates duplicate each parent ex column (broadcast AP).
  - c/h math on DVE in bf16; h written transposed to DRAM bf16, host
    un-transposes and upcasts to f32.
"""

import numpy as np
import ml_dtypes

import concourse.bass as bass
import concourse.mybir as mybir
import concourse.tile as tile
from concourse import bacc
from concourse.bass_utils import run_bass_kernel_spmd

F32 = mybir.dt.float32
F32R = mybir.dt.float32r
BF16 = mybir.dt.bfloat16
F8 = mybir.dt.float8e4
DR = mybir.MatmulPerfMode.DoubleRow
AF = mybir.ActivationFunctionType

B, D, DIM = 32, 11, 300
N = 2**D - 1          # 2047
CORES = 8
BL = B // CORES       # 4 trees per core
NTOT = BL * N         # 8188 columns per core
KS = [128, 128, 44]   # feature chunks of 300
NB = 512
GATES = ("i", "o", "u", "f")

# level -> column offset in the level-major layout (leaves first)
OFF = {}
_o = 0
for _l in range(D - 1, -1, -1):
    OFF[_l] = _o
    _o += BL * (1 << _l)
TAIL_LV = 7           # levels <= TAIL_LV use the resident ex tile
TOFF = OFF[TAIL_LV]   # 7168
TCOLS = NTOT - TOFF   # 1020

_NC_CACHE = []


def _cols(l):
    return BL * (1 << l)


def _build():
    nc = bacc.Bacc("TRN2", target_bir_lowering=False, debug=False,
                   num_devices=CORES)
    exT = nc.dram_tensor("ext", [128, 3, NTOT], BF16, kind="ExternalInput")
    WXS = nc.dram_tensor("wxs", [128, 3, 12, 128], BF16, kind="ExternalInput")
    WHS = nc.dram_tensor("whs", [128, 3, 12, 128], BF16, kind="ExternalInput")
    WHA = nc.dram_tensor("wha", [128, 2, 9, 128], F8, kind="ExternalInput")
    WHB = nc.dram_tensor("whb", [44, 2, 9, 128], F8, kind="ExternalInput")
    houtT = nc.dram_tensor("houtt", [128, 3, NTOT], BF16,
                           kind="ExternalOutput")

    with tile.TileContext(nc) as tc:
        import contextlib
        with contextlib.ExitStack() as ctx:
            sb = ctx.enter_context(tc.tile_pool(name="sb", bufs=1))
            exp = ctx.enter_context(tc.tile_pool(name="exp", bufs=6))
            hsp = ctx.enter_context(tc.tile_pool(name="hsp", bufs=2))
            gp = ctx.enter_context(tc.tile_pool(name="gp", bufs=8))
            fcp = ctx.enter_context(tc.tile_pool(name="fcp", bufs=2))
            stp = ctx.enter_context(tc.tile_pool(name="stp", bufs=1))
            psum = ctx.enter_context(
                tc.tile_pool(name="psum", bufs=2, space="PSUM"))

            wx_t = sb.tile([128, 3, 12, 128], BF16, name="wx_t")
            wh_t = sb.tile([128, 3, 12, 128], BF16, name="wh_t")
            wha_t = sb.tile([128, 2, 9, 128], F8, name="wha_t")
            whb_t = sb.tile([44, 2, 9, 128], F8, name="whb_t")
            ex_tail = sb.tile([128, 3, TCOLS], BF16, name="ex_tail")

            # fp8 child-sum tiles for the iou h-side (DoubleRow rhs layout).
            # hsB's second k-tile rides against zero weight rows; memset it
            # once per buffer so stale bytes can never decode as NaN.
            hsA_t = [sb.tile([128, 2, NB], F8, name=f"hsA{j}")
                     for j in range(2)]
            hsB_t = [sb.tile([44, 2, NB], F8, name=f"hsB{j}")
                     for j in range(2)]
            for j in range(2):
                nc.gpsimd.memset(hsB_t[j][:, :, :], 0.0)

            # persistent per-level h/c state (bf16, feature-transposed)
            st_h = {l: stp.tile([128, 3, _cols(l)], BF16, tag=f"sh{l}",
                                name=f"sh{l}") for l in range(D)}
            st_c = {l: stp.tile([128, 3, _cols(l)], BF16, tag=f"sc{l}",
                                name=f"sc{l}") for l in range(D)}

            def gate_x_matmuls(pG, g, ex, e0, nb, start, stop):
                """Accumulate x-side pre-act for gate g over cols [e0,e0+nb)."""
                for m in range(3):
                    ms = KS[m]
                    for k in range(3):
                        kx = KS[k] + (1 if k == 2 else 0)
                        nc.tensor.matmul(
                            pG[0:ms, m, 0:nb],
                            wx_t[0:kx, k, 3 * GATES.index(g) + m, 0:ms],
                            ex[0:kx, k, e0:e0 + nb],
                            start=(start and k == 0),
                            stop=(stop and k == 2))

            def gate_h_matmuls(pG, g, hsA, hsB, nb, stop):
                gi = GATES.index(g)
                for m in range(3):
                    ms = KS[m]
                    nc.tensor.matmul(
                        pG[0:ms, m, 0:nb],
                        wha_t[0:128, 0:2, 3 * gi + m, 0:ms],
                        hsA[0:128, 0:2, 0:nb],
                        start=False, stop=False, perf_mode=DR)
                    nc.tensor.matmul(
                        pG[0:ms, m, 0:nb],
                        whb_t[0:44, 0:2, 3 * gi + m, 0:ms],
                        hsB[0:44, 0:2, 0:nb],
                        start=False, stop=(stop and m == 2), perf_mode=DR)

            def ex_for(l, c0, nb):
                """Return (tile, base offset) holding ex cols of level l."""
                if l <= TAIL_LV:
                    return ex_tail, OFF[l] - TOFF + c0
                t = exp.tile([128, 3, NB], BF16, tag="ex")
                nc.sync.dma_start(
                    out=t[:, :, 0:nb],
                    in_=exT[:, :, OFF[l] + c0:OFF[l] + c0 + nb])
                return t, 0

            def blocks_of(l):
                cols = _cols(l)
                if cols > NB:
                    return NB
                return max(cols // 2, 2) if cols >= 16 else cols

            # ---------------- leaves (level 10) ----------------
            l = D - 1
            ex0, e00 = ex_for(l, 0, NB)
            nc.sync.dma_start(out=wx_t[:, :, :, :], in_=WXS[:, :, :, :])
            first_leaf = True
            for c0 in range(0, _cols(l), NB):
                nb = min(NB, _cols(l) - c0)
                if first_leaf:
                    ex, e0 = ex0, e00
                    first_leaf = False
                else:
                    ex, e0 = ex_for(l, c0, nb)
                sg = {}
                for g, fn in (("i", AF.Sigmoid), ("u", AF.Tanh),
                              ("o", AF.Sigmoid)):
                    pG = psum.tile([128, 3, NB], F32, tag="big")
                    gate_x_matmuls(pG, g, ex, e0, nb, True, True)
                    gt = gp.tile([128, 3, NB], BF16, tag="g")
                    nc.scalar.activation(gt[:, :, 0:nb], pG[:, :, 0:nb], fn)
                    sg[g] = gt
                    if g == "u":
                        cc = st_c[l][:, :, c0:c0 + nb]
                        nc.vector.tensor_mul(cc, sg["i"][:, :, 0:nb],
                                             sg["u"][:, :, 0:nb])
                if c0 == 0:
                    nc.sync.dma_start(out=wh_t[:, :, :, :],
                                      in_=WHS[:, :, :, :])
                    nc.sync.dma_start(out=wha_t[:, :, :, :],
                                      in_=WHA[:, :, :, :])
                    nc.sync.dma_start(out=whb_t[:, :, :, :],
                                      in_=WHB[:, :, :, :])
                th = gp.tile([128, 3, NB], BF16, tag="g")
                nc.scalar.activation(th[:, :, 0:nb], cc, AF.Tanh)
                nc.vector.tensor_mul(st_h[l][:, :, c0:c0 + nb],
                                     sg["o"][:, :, 0:nb], th[:, :, 0:nb])
            nc.sync.dma_start(out=ex_tail[:, :, :],
                              in_=exT[:, :, TOFF:TOFF + TCOLS])
            nc.gpsimd.dma_start(out=houtT[:, :, OFF[l]:OFF[l] + _cols(l)],
                                in_=st_h[l][:, :, :])

            # ---------------- internal levels 9..0 ----------------
            iblk = [0]
            for l in range(D - 2, -1, -1):
                cols = _cols(l)
                nbl = blocks_of(l)
                for c0 in range(0, cols, nbl):
                    nb = min(nbl, cols - c0)
                    fs = min(2 * nb, NB)     # child-block width
                    nsub = (2 * nb) // fs
                    ex, e0 = ex_for(l, c0, nb)
                    ch_h = st_h[l + 1]
                    ch_c = st_c[l + 1]

                    # forget gates first: they depend only on child h/c
                    pFs = []
                    for s in range(nsub):
                        ch0 = 2 * c0 + s * fs
                        p0 = s * fs // 2
                        pF = psum.tile([128, 3, NB], F32, tag="big")
                        for m in range(3):
                            ms = KS[m]
                            for k in range(3):
                                kx = KS[k] + (1 if k == 2 else 0)
                                dup = ex[0:kx, k, e0 + p0:e0 + p0 + fs // 2] \
                                    .unsqueeze(2).broadcast_to(
                                        [kx, fs // 2, 2])
                                nc.tensor.matmul(
                                    pF[0:ms, m, 0:fs],
                                    wx_t[0:kx, k, 9 + m, 0:ms],
                                    dup, start=(k == 0), stop=False)
                            for k in range(3):
                                nc.tensor.matmul(
                                    pF[0:ms, m, 0:fs],
                                    wh_t[0:KS[k], k, 9 + m, 0:ms],
                                    ch_h[0:KS[k], k, ch0:ch0 + fs],
                                    start=False, stop=(k == 2))
                        fg = gp.tile([128, 3, NB], BF16, tag="g")
                        nc.scalar.activation(fg[:, :, 0:fs], pF[:, :, 0:fs],
                                             AF.Sigmoid)
                        pFs.append((fg, s))

                    # child sum h1+h2 -> fp8 DR rhs tiles
                    hsA = hsA_t[iblk[0] % 2]
                    hsB = hsB_t[iblk[0] % 2]
                    iblk[0] += 1
                    pairA = ch_h[:, 0:2, 2 * c0:2 * c0 + 2 * nb].rearrange(
                        "p c (n two) -> p c n two", two=2)
                    nc.vector.tensor_add(hsA[:, :, 0:nb],
                                         pairA[:, :, :, 0], pairA[:, :, :, 1])
                    pairB = ch_h[0:44, 2, 2 * c0:2 * c0 + 2 * nb].rearrange(
                        "p (n two) -> p n two", two=2)
                    nc.vector.tensor_add(hsB[0:44, 0, 0:nb],
                                         pairB[:, :, 0], pairB[:, :, 1])

                    sg = {}
                    cc = st_c[l][:, :, c0:c0 + nb]
                    for g, fn in (("i", AF.Sigmoid), ("u", AF.Tanh),
                                  ("o", AF.Sigmoid)):
                        pG = psum.tile([128, 3, NB], F32, tag="big")
                        gate_x_matmuls(pG, g, ex, e0, nb, True, False)
                        gate_h_matmuls(pG, g, hsA, hsB, nb, True)
                        gt = gp.tile([128, 3, NB], BF16, tag="g")
                        nc.scalar.activation(gt[:, :, 0:nb], pG[:, :, 0:nb],
                                             fn)
                        sg[g] = gt
                        if g == "u":
                            nc.vector.tensor_mul(cc, sg["i"][:, :, 0:nb],
                                                 sg["u"][:, :, 0:nb])

                    for fg, s in pFs:
                        ch0 = 2 * c0 + s * fs
                        p0 = s * fs // 2
                        fc = fcp.tile([128, 3, NB], BF16, tag="fc")
                        nc.vector.tensor_mul(fc[:, :, 0:fs],
                                             fg[:, :, 0:fs],
                                             ch_c[:, :, ch0:ch0 + fs])
                        fpair = fc[:, :, 0:fs].rearrange(
                            "p c (n two) -> p c n two", two=2)
                        ccs = cc[:, :, p0:p0 + fs // 2] if nsub > 1 else cc
                        nc.vector.tensor_add(ccs, ccs, fpair[:, :, :, 0])
                        nc.vector.tensor_add(ccs, ccs, fpair[:, :, :, 1])

                    th = gp.tile([128, 3, NB], BF16, tag="g")
                    nc.scalar.activation(th[:, :, 0:nb], cc, AF.Tanh)
                    nc.vector.tensor_mul(st_h[l][:, :, c0:c0 + nb],
                                         sg["o"][:, :, 0:nb], th[:, :, 0:nb])
                nc.gpsimd.dma_start(out=houtT[:, :, OFF[l]:OFF[l] + cols],
                                    in_=st_h[l][:, :, :])
    nc.compile()
    return nc


def _prep_inputs(embs, Wx, bx, Wh, bh):
    """Host-side: transposed bf16 ex + weight slabs."""
    bf = ml_dtypes.bfloat16
    ex = np.zeros((CORES, 128, 3, NTOT), dtype=bf)
    e32 = np.asarray(embs, np.float32)
    for c in range(CORES):
        ec = e32[BL * c:BL * (c + 1)]          # [BL, N, 300]
        for l in range(D - 1, -1, -1):
            n0, n1 = (1 << l) - 1, (1 << (l + 1)) - 1
            T = ec[:, n0:n1, :].reshape(BL * (1 << l), DIM).T  # [300, cols]
            o0 = OFF[l]
            nbl = BL * (1 << l)
            ex[c, :, 0, o0:o0 + nbl] = T[0:128]
            ex[c, :, 1, o0:o0 + nbl] = T[128:256]
            ex[c, 0:44, 2, o0:o0 + nbl] = T[256:300]
            ex[c, 44, 2, o0:o0 + nbl] = 1.0

    def slab(Ws, biases):
        out = np.zeros((128, 3, 12, 128), dtype=bf)
        for gi in range(4):
            W = np.asarray(Ws[gi], np.float32)
            for m in range(3):
                ms = KS[m]
                blk = W[:, 128 * m:128 * m + ms]           # [300, ms]
                g = 3 * gi + m
                out[:, 0, g, 0:ms] = blk[0:128]
                out[:, 1, g, 0:ms] = blk[128:256]
                out[0:44, 2, g, 0:ms] = blk[256:300]
                if biases is not None:
                    out[44, 2, g, 0:ms] = biases[gi][128 * m:128 * m + ms]
        return out

    wxs = slab(Wx, bx)
    whs = slab(Wh, None)

    f8 = ml_dtypes.float8_e4m3
    wha = np.zeros((128, 2, 9, 128), dtype=f8)
    whb = np.zeros((44, 2, 9, 128), dtype=f8)
    for gi in range(3):                      # i, o, u only
        W = np.asarray(Wh[gi], np.float32)
        for m in range(3):
            ms = KS[m]
            blk = W[:, 128 * m:128 * m + ms]
            g = 3 * gi + m
            wha[:, 0, g, 0:ms] = blk[0:128]
            wha[:, 1, g, 0:ms] = blk[128:256]
            whb[:, 0, g, 0:ms] = blk[256:300]
    return ex, wxs, whs, wha, whb


def kernel(embs, Wix, bix, Wih, bih, Wfx, bfx, Wfh, bfh,
           Wox, box, Woh, boh, Wux, bux, Wuh, buh):
    if not _NC_CACHE:
        _NC_CACHE.append(_build())
    nc = _NC_CACHE[0]

    bxs = [np.asarray(bix) + np.asarray(bih),
           np.asarray(box) + np.asarray(boh),
           np.asarray(bux) + np.asarray(buh),
           np.asarray(bfx) + np.asarray(bfh)]
    ex, wxs, whs, wha, whb = _prep_inputs(embs, [Wix, Wox, Wux, Wfx], bxs,
                                          [Wih, Woh, Wuh, Wfh], None)

    in_maps = [{"ext": ex[c], "wxs": wxs, "whs": whs, "wha": wha, "whb": whb}
               for c in range(CORES)]
    res = run_bass_kernel_spmd(nc, in_maps, list(range(CORES)))

    hout = np.zeros((B, N, DIM), np.float32)
    for c in range(CORES):
        ht = np.asarray(res.results[c]["houtt"], np.float32)  # [128,3,NTOT]
        for l in range(D):
            n0, n1 = (1 << l) - 1, (1 << (l + 1)) - 1
            nbl = BL * (1 << l)
            o0 = OFF[l]
            Hl = np.concatenate(
                [ht[0:128, 0, o0:o0 + nbl], ht[0:128, 1, o0:o0 + nbl],
                 ht[0:44, 2, o0:o0 + nbl]], axis=0)         # [300, cols]
            hout[BL * c:BL * (c + 1), n0:n1, :] = \
                Hl.T.reshape(BL, 1 << l, DIM)
    return hout


# revision 5
# speedup vs baseline: 1.4698x; 1.0037x over previous
"""Child-Sum TreeLSTM over complete binary trees — Trainium2 Bass kernel (v2).

Sharding: data-parallel over batch B=32 across 8 cores (4 trees/core),
weights replicated.

v2 design (vs v1): all-bf16 datapath, zero on-device transposes.
  - Host pre-transposes embs into feature-major layout [128, 3, 8188] bf16
    with a baked ones-row (feature slot 300) riding the k=2 chunk; the
    combined bias (bx+bh) is a 45th weight row.
  - Weight slabs WXS/WHS [128, 3kc, 12grp, 128] bf16 (gate x m-chunk grid).
  - All h/c state for all 11 levels stays resident in SBUF as bf16
    [128, 3, cols] tiles — no DRAM spills.
  - Gate pre-acts accumulate in PSUM f32 over 3 x-chunks + 3 h-chunks;
    one Act instruction per gate evacuates PSUM -> SBUF bf16 with sigma/tanh.
  - Per-child forget gates duplicate each parent ex column (broadcast AP).
  - c/h math on DVE in bf16; h written transposed to DRAM bf16, host
    un-transposes and upcasts to f32.
"""

import numpy as np
import ml_dtypes

import concourse.bass as bass
import concourse.mybir as mybir
import concourse.tile as tile
from concourse import bacc
from concourse.bass_utils import run_bass_kernel_spmd

F32 = mybir.dt.float32
F32R = mybir.dt.float32r
BF16 = mybir.dt.bfloat16
F8 = mybir.dt.float8e4
DR = mybir.MatmulPerfMode.DoubleRow
AF = mybir.ActivationFunctionType

B, D, DIM = 32, 11, 300
N = 2**D - 1          # 2047
CORES = 8
BL = B // CORES       # 4 trees per core
NTOT = BL * N         # 8188 columns per core
KS = [128, 128, 44]   # feature chunks of 300
NB = 512
GATES = ("i", "o", "u", "f")

# level -> column offset in the level-major layout (leaves first)
OFF = {}
_o = 0
for _l in range(D - 1, -1, -1):
    OFF[_l] = _o
    _o += BL * (1 << _l)
TAIL_LV = 7           # levels <= TAIL_LV use the resident ex tile
TOFF = OFF[TAIL_LV]   # 7168
TCOLS = NTOT - TOFF   # 1020

_NC_CACHE = []


def _cols(l):
    return BL * (1 << l)


def _build():
    nc = bacc.Bacc("TRN2", target_bir_lowering=False, debug=False,
                   num_devices=CORES)
    exT = nc.dram_tensor("ext", [128, 3, NTOT], BF16, kind="ExternalInput")
    WXS = nc.dram_tensor("wxs", [128, 3, 12, 128], BF16, kind="ExternalInput")
    WHA = nc.dram_tensor("wha", [128, 2, 12, 128], F8, kind="ExternalInput")
    WHB = nc.dram_tensor("whb", [44, 2, 12, 128], F8, kind="ExternalInput")
    houtT = nc.dram_tensor("houtt", [128, 3, NTOT], BF16,
                           kind="ExternalOutput")

    with tile.TileContext(nc) as tc:
        import contextlib
        with contextlib.ExitStack() as ctx:
            sb = ctx.enter_context(tc.tile_pool(name="sb", bufs=1))
            exp = ctx.enter_context(tc.tile_pool(name="exp", bufs=6))
            hsp = ctx.enter_context(tc.tile_pool(name="hsp", bufs=2))
            gp = ctx.enter_context(tc.tile_pool(name="gp", bufs=8))
            fcp = ctx.enter_context(tc.tile_pool(name="fcp", bufs=2))
            stp = ctx.enter_context(tc.tile_pool(name="stp", bufs=1))
            psum = ctx.enter_context(
                tc.tile_pool(name="psum", bufs=2, space="PSUM"))

            wx_t = sb.tile([128, 3, 12, 128], BF16, name="wx_t")
            wha_t = sb.tile([128, 2, 12, 128], F8, name="wha_t")
            whb_t = sb.tile([44, 2, 12, 128], F8, name="whb_t")
            ex_tail = sb.tile([128, 3, TCOLS], BF16, name="ex_tail")

            # fp8 child-sum tiles: chunk dim doubles as the DR k-tile dim.
            # chunk 3 rides against zero weight rows; memset once per buffer
            # so stale bytes can never decode as NaN.
            hs8_t = [sb.tile([128, 4, NB], F8, name=f"hs8{j}")
                     for j in range(2)]
            for j in range(2):
                nc.gpsimd.memset(hs8_t[j][:, 3, :], 0.0)

            # persistent per-level h/c state (bf16, feature-transposed)
            st_h = {l: stp.tile([128, 3, _cols(l)], BF16, tag=f"sh{l}",
                                name=f"sh{l}") for l in range(D)}
            st_c = {l: stp.tile([128, 3, _cols(l)], BF16, tag=f"sc{l}",
                                name=f"sc{l}") for l in range(D)}
            # fp8 copies of h for the forget-gate DR matmuls (chunk 3 zero)
            st_h8 = {l: stp.tile([128, 4, _cols(l)], F8, tag=f"sq{l}",
                                 name=f"sq{l}") for l in range(1, D)}
            for l in range(1, D):
                nc.gpsimd.memset(st_h8[l][:, 3, :], 0.0)

            def gate_x_matmuls(pG, g, ex, e0, nb, start, stop):
                """Accumulate x-side pre-act for gate g over cols [e0,e0+nb)."""
                for m in range(3):
                    ms = KS[m]
                    for k in range(3):
                        kx = KS[k] + (1 if k == 2 else 0)
                        nc.tensor.matmul(
                            pG[0:ms, m, 0:nb],
                            wx_t[0:kx, k, 3 * GATES.index(g) + m, 0:ms],
                            ex[0:kx, k, e0:e0 + nb],
                            start=(start and k == 0),
                            stop=(stop and k == 2))

            def gate_h_matmuls(pG, g, h8, h0, nb, start, stop):
                gi = GATES.index(g)
                for m in range(3):
                    ms = KS[m]
                    nc.tensor.matmul(
                        pG[0:ms, m, 0:nb],
                        wha_t[0:128, 0:2, 3 * gi + m, 0:ms],
                        h8[0:128, 0:2, h0:h0 + nb],
                        start=(start and m == 0), stop=False, perf_mode=DR)
                    nc.tensor.matmul(
                        pG[0:ms, m, 0:nb],
                        whb_t[0:44, 0:2, 3 * gi + m, 0:ms],
                        h8[0:44, 2:4, h0:h0 + nb],
                        start=False, stop=(stop and m == 2), perf_mode=DR)

            def ex_for(l, c0, nb):
                """Return (tile, base offset) holding ex cols of level l."""
                if l <= TAIL_LV:
                    return ex_tail, OFF[l] - TOFF + c0
                t = exp.tile([128, 3, NB], BF16, tag="ex")
                nc.sync.dma_start(
                    out=t[:, :, 0:nb],
                    in_=exT[:, :, OFF[l] + c0:OFF[l] + c0 + nb])
                return t, 0

            def blocks_of(l):
                cols = _cols(l)
                if cols > NB:
                    return NB
                return max(cols // 2, 2) if cols >= 16 else cols

            # ---------------- leaves (level 10) ----------------
            l = D - 1
            ex0, e00 = ex_for(l, 0, NB)
            nc.sync.dma_start(out=wx_t[:, :, :, :], in_=WXS[:, :, :, :])
            first_leaf = True
            for c0 in range(0, _cols(l), NB):
                nb = min(NB, _cols(l) - c0)
                if first_leaf:
                    ex, e0 = ex0, e00
                    first_leaf = False
                else:
                    ex, e0 = ex_for(l, c0, nb)
                sg = {}
                for g, fn in (("i", AF.Sigmoid), ("u", AF.Tanh),
                              ("o", AF.Sigmoid)):
                    pG = psum.tile([128, 3, NB], F32, tag="big")
                    gate_x_matmuls(pG, g, ex, e0, nb, True, True)
                    gt = gp.tile([128, 3, NB], BF16, tag="g")
                    nc.scalar.activation(gt[:, :, 0:nb], pG[:, :, 0:nb], fn)
                    sg[g] = gt
                    if g == "u":
                        cc = st_c[l][:, :, c0:c0 + nb]
                        nc.vector.tensor_mul(cc, sg["i"][:, :, 0:nb],
                                             sg["u"][:, :, 0:nb])
                if c0 == 0:
                    nc.sync.dma_start(out=wha_t[:, :, :, :],
                                      in_=WHA[:, :, :, :])
                    nc.sync.dma_start(out=whb_t[:, :, :, :],
                                      in_=WHB[:, :, :, :])
                th = gp.tile([128, 3, NB], BF16, tag="g")
                nc.scalar.activation(th[:, :, 0:nb], cc, AF.Tanh)
                nc.vector.tensor_mul(st_h8[l][:, 0:3, c0:c0 + nb],
                                     sg["o"][:, :, 0:nb], th[:, :, 0:nb])
                nc.vector.tensor_mul(st_h[l][:, :, c0:c0 + nb],
                                     sg["o"][:, :, 0:nb], th[:, :, 0:nb])
            nc.sync.dma_start(out=ex_tail[:, :, :],
                              in_=exT[:, :, TOFF:TOFF + TCOLS])
            nc.gpsimd.dma_start(out=houtT[:, :, OFF[l]:OFF[l] + _cols(l)],
                                in_=st_h[l][:, :, :])

            # ---------------- internal levels 9..0 ----------------
            iblk = [0]
            for l in range(D - 2, -1, -1):
                cols = _cols(l)
                nbl = blocks_of(l)
                for c0 in range(0, cols, nbl):
                    nb = min(nbl, cols - c0)
                    fs = min(2 * nb, NB)     # child-block width
                    nsub = (2 * nb) // fs
                    ex, e0 = ex_for(l, c0, nb)
                    ch_h = st_h[l + 1]
                    ch_c = st_c[l + 1]

                    # forget gates first: they depend only on child h/c
                    pFs = []
                    for s in range(nsub):
                        ch0 = 2 * c0 + s * fs
                        p0 = s * fs // 2
                        pF = psum.tile([128, 3, NB], F32, tag="big")
                        for m in range(3):
                            ms = KS[m]
                            for k in range(3):
                                kx = KS[k] + (1 if k == 2 else 0)
                                dup = ex[0:kx, k, e0 + p0:e0 + p0 + fs // 2] \
                                    .unsqueeze(2).broadcast_to(
                                        [kx, fs // 2, 2])
                                nc.tensor.matmul(
                                    pF[0:ms, m, 0:fs],
                                    wx_t[0:kx, k, 9 + m, 0:ms],
                                    dup, start=(k == 0), stop=False)
                        gate_h_matmuls(pF, "f", st_h8[l + 1], ch0, fs,
                                       False, True)
                        fg = gp.tile([128, 3, NB], BF16, tag="g")
                        nc.scalar.activation(fg[:, :, 0:fs], pF[:, :, 0:fs],
                                             AF.Sigmoid)
                        pFs.append((fg, s))

                    # child sum h1+h2 -> fp8 DR rhs tile
                    hs8 = hs8_t[iblk[0] % 2]
                    iblk[0] += 1
                    pair = ch_h[:, :, 2 * c0:2 * c0 + 2 * nb].rearrange(
                        "p c (n two) -> p c n two", two=2)
                    nc.vector.tensor_add(hs8[:, 0:3, 0:nb],
                                         pair[:, :, :, 0], pair[:, :, :, 1])

                    sg = {}
                    cc = st_c[l][:, :, c0:c0 + nb]
                    for g, fn in (("i", AF.Sigmoid), ("u", AF.Tanh),
                                  ("o", AF.Sigmoid)):
                        pG = psum.tile([128, 3, NB], F32, tag="big")
                        gate_x_matmuls(pG, g, ex, e0, nb, True, False)
                        gate_h_matmuls(pG, g, hs8, 0, nb, False, True)
                        gt = gp.tile([128, 3, NB], BF16, tag="g")
                        nc.scalar.activation(gt[:, :, 0:nb], pG[:, :, 0:nb],
                                             fn)
                        sg[g] = gt
                        if g == "u":
                            nc.vector.tensor_mul(cc, sg["i"][:, :, 0:nb],
                                                 sg["u"][:, :, 0:nb])

                    for fg, s in pFs:
                        ch0 = 2 * c0 + s * fs
                        p0 = s * fs // 2
                        fc = fcp.tile([128, 3, NB], BF16, tag="fc")
                        nc.vector.tensor_mul(fc[:, :, 0:fs],
                                             fg[:, :, 0:fs],
                                             ch_c[:, :, ch0:ch0 + fs])
                        fpair = fc[:, :, 0:fs].rearrange(
                            "p c (n two) -> p c n two", two=2)
                        ccs = cc[:, :, p0:p0 + fs // 2] if nsub > 1 else cc
                        nc.vector.tensor_add(ccs, ccs, fpair[:, :, :, 0])
                        nc.vector.tensor_add(ccs, ccs, fpair[:, :, :, 1])

                    th = gp.tile([128, 3, NB], BF16, tag="g")
                    nc.scalar.activation(th[:, :, 0:nb], cc, AF.Tanh)
                    if l >= 1:
                        nc.vector.tensor_mul(st_h8[l][:, 0:3, c0:c0 + nb],
                                             sg["o"][:, :, 0:nb],
                                             th[:, :, 0:nb])
                    nc.vector.tensor_mul(st_h[l][:, :, c0:c0 + nb],
                                         sg["o"][:, :, 0:nb], th[:, :, 0:nb])
                nc.gpsimd.dma_start(out=houtT[:, :, OFF[l]:OFF[l] + cols],
                                    in_=st_h[l][:, :, :])
    nc.compile()
    return nc


def _prep_inputs(embs, Wx, bx, Wh, bh):
    """Host-side: transposed bf16 ex + weight slabs."""
    bf = ml_dtypes.bfloat16
    ex = np.zeros((CORES, 128, 3, NTOT), dtype=bf)
    e32 = np.asarray(embs, np.float32)
    for c in range(CORES):
        ec = e32[BL * c:BL * (c + 1)]          # [BL, N, 300]
        for l in range(D - 1, -1, -1):
            n0, n1 = (1 << l) - 1, (1 << (l + 1)) - 1
            T = ec[:, n0:n1, :].reshape(BL * (1 << l), DIM).T  # [300, cols]
            o0 = OFF[l]
            nbl = BL * (1 << l)
            ex[c, :, 0, o0:o0 + nbl] = T[0:128]
            ex[c, :, 1, o0:o0 + nbl] = T[128:256]
            ex[c, 0:44, 2, o0:o0 + nbl] = T[256:300]
            ex[c, 44, 2, o0:o0 + nbl] = 1.0

    def slab(Ws, biases):
        out = np.zeros((128, 3, 12, 128), dtype=bf)
        for gi in range(4):
            W = np.asarray(Ws[gi], np.float32)
            for m in range(3):
                ms = KS[m]
                blk = W[:, 128 * m:128 * m + ms]           # [300, ms]
                g = 3 * gi + m
                out[:, 0, g, 0:ms] = blk[0:128]
                out[:, 1, g, 0:ms] = blk[128:256]
                out[0:44, 2, g, 0:ms] = blk[256:300]
                if biases is not None:
                    out[44, 2, g, 0:ms] = biases[gi][128 * m:128 * m + ms]
        return out

    wxs = slab(Wx, bx)

    f8 = ml_dtypes.float8_e4m3
    wha = np.zeros((128, 2, 12, 128), dtype=f8)
    whb = np.zeros((44, 2, 12, 128), dtype=f8)
    for gi in range(4):                      # i, o, u, f
        W = np.asarray(Wh[gi], np.float32)
        for m in range(3):
            ms = KS[m]
            blk = W[:, 128 * m:128 * m + ms]
            g = 3 * gi + m
            wha[:, 0, g, 0:ms] = blk[0:128]
            wha[:, 1, g, 0:ms] = blk[128:256]
            whb[:, 0, g, 0:ms] = blk[256:300]
    return ex, wxs, wha, whb


def kernel(embs, Wix, bix, Wih, bih, Wfx, bfx, Wfh, bfh,
           Wox, box, Woh, boh, Wux, bux, Wuh, buh):
    if not _NC_CACHE:
        _NC_CACHE.append(_build())
    nc = _NC_CACHE[0]

    bxs = [np.asarray(bix) + np.asarray(bih),
           np.asarray(box) + np.asarray(boh),
           np.asarray(bux) + np.asarray(buh),
           np.asarray(bfx) + np.asarray(bfh)]
    ex, wxs, wha, whb = _prep_inputs(embs, [Wix, Wox, Wux, Wfx], bxs,
                                     [Wih, Woh, Wuh, Wfh], None)

    in_maps = [{"ext": ex[c], "wxs": wxs, "wha": wha, "whb": whb}
               for c in range(CORES)]
    res = run_bass_kernel_spmd(nc, in_maps, list(range(CORES)))

    hout = np.zeros((B, N, DIM), np.float32)
    for c in range(CORES):
        ht = np.asarray(res.results[c]["houtt"], np.float32)  # [128,3,NTOT]
        for l in range(D):
            n0, n1 = (1 << l) - 1, (1 << (l + 1)) - 1
            nbl = BL * (1 << l)
            o0 = OFF[l]
            Hl = np.concatenate(
                [ht[0:128, 0, o0:o0 + nbl], ht[0:128, 1, o0:o0 + nbl],
                 ht[0:44, 2, o0:o0 + nbl]], axis=0)         # [300, cols]
            hout[BL * c:BL * (c + 1), n0:n1, :] = \
                Hl.T.reshape(BL, 1 << l, DIM)
    return hout
